# revision 1
# baseline (speedup 1.0000x reference)
"""GBST pooling kernel for Trainium2 (Bass/Tile), 8-core data-parallel.

Problem (per batch b, data-parallel over 8 cores):
    x [T=8192, D=512] f32, W [K=4, D] f32
    pooled_k[t] = mean(x[t:t+k]) (valid window, zero-padded tail)
    scores[t,k] = <pooled_k[t], W[k]>;  w = softmax_k(scores)
    out[t] = sum_k w[t,k] * pooled_k[t]

Kernel strategy: time is tiled into 125-output-column tiles (each consuming 128
x rows, 3-row overlap), processed in groups of NB tiles so every DMA is
amortized across the group (HWDGE has ~625ns serialized overhead per DMA):
    - one merged x load per group [128, NB, 512]
    - per tile: 4 PE transposes -> xT; 4 accumulating PE matmuls -> u[t,k] =
      <x[t], W[k]>; DVE copy u -> u_big
    - one u write + 3 shifted reads per group (DRAM roundtrip implements the
      partition shifts needed for the sliding-window score sums)
    - per tile: score/softmax/coefficient smalls on DVE+ACT -> C into c_big
    - one staircase write c_big -> A_dram slot per group: band matrix
      A[t, 128b + t'] = c_{t-t'}[t'] (slots pre-zeroed once; off-band cells
      stay zero forever since staircase cells sit at identical flat offsets)
    - one A readback per group; per tile one fp32 PE matmul
      out[t', d] = sum_t A[t, t'] x[t, d] does the entire pooling+blend
    - PSUM -> SBUF copies split ACT/DVE; one merged out store per group
"""

import sys

if "/opt/trn_rl_repo" not in sys.path:
    sys.path.insert(0, "/opt/trn_rl_repo")

from contextlib import ExitStack

import numpy as np

import concourse.bass as bass
import concourse.bacc as bacc_mod
import concourse.mybir as mybir
import concourse.tile as tile
from concourse.masks import make_identity

F32 = mybir.dt.float32
F32R = mybir.dt.float32r
USE_F32R_BLEND = False
USE_F32R_TRANSPOSE = False

B, T, D, K = 8, 8192, 512, 4
N_CORES = 8
TP = 125          # output columns per tile (128 - (K-1))
NB = 8            # tiles per DMA-batched group
NSLOT = 4         # rotating DRAM scratch slots (group-sized)


def build_nc(t_total=T, d_total=D, k_scales=K, nb=NB):
    nc = bacc_mod.Bacc(None, target_bir_lowering=False)
    x_in = nc.dram_tensor("x", (t_total, d_total), F32, kind="ExternalInput")
    w_in = nc.dram_tensor("W", (k_scales, d_total), F32, kind="ExternalInput")
    out_dram = nc.dram_tensor("out", (t_total, d_total), F32, kind="ExternalOutput")

    n_tiles = (t_total + TP - 1) // TP
    n_groups = (n_tiles + nb - 1) // nb
    n_chunks = d_total // 128
    acols = 128 * nb                    # A-slot columns
    half = d_total // 2

    with tile.TileContext(nc) as tc, ExitStack() as ctx:
        consts = ctx.enter_context(tc.tile_pool(name="consts", bufs=1))
        xpool = ctx.enter_context(tc.tile_pool(name="xpool", bufs=4))
        xtpool = ctx.enter_context(tc.tile_pool(name="xtpool", bufs=4))
        upool = ctx.enter_context(tc.tile_pool(name="upool", bufs=3))
        smalls = ctx.enter_context(tc.tile_pool(name="smalls", bufs=3 * nb))
        cpool = ctx.enter_context(tc.tile_pool(name="cpool", bufs=3))
        apool = ctx.enter_context(tc.tile_pool(name="apool", bufs=3))
        opool = ctx.enter_context(tc.tile_pool(name="opool", bufs=4))
        ppool_t = ctx.enter_context(tc.tile_pool(name="ppool_t", bufs=3, space="PSUM"))
        ppool_u = ctx.enter_context(tc.tile_pool(name="ppool_u", bufs=2, space="PSUM"))
        ppool_o = ctx.enter_context(tc.tile_pool(name="ppool_o", bufs=3, space="PSUM"))
        dram = ctx.enter_context(tc.tile_pool(name="dram", bufs=1, space="DRAM"))

        # ---- constants ----
        identity = consts.tile([128, 128], F32)
        make_identity(nc, identity)

        # W_sb[p, c, k] = W[k, 128c + p]
        w_sb = consts.tile([128, n_chunks, k_scales], F32)
        for c in range(n_chunks):
            w_src = bass.AP(
                tensor=w_in.ap().tensor,
                offset=c * 128,
                ap=[[1, 128], [d_total, k_scales]],
            )
            nc.sync.dma_start(out=w_sb[:, c, :], in_=w_src)

        invk = consts.tile([128, k_scales], F32)
        for k in range(k_scales):
            nc.gpsimd.memset(invk[:, k : k + 1], 1.0 / (k + 1))
        for c in range(n_chunks):
            nc.vector.tensor_mul(w_sb[:, c, :], w_sb[:, c, :], invk[:, :])

        zero_sb = consts.tile([128, acols], F32)
        nc.gpsimd.memset(zero_sb[:], 0.0)

        # ---- DRAM scratch: staircase A slots + u roundtrip slots ----
        a_slots = [
            dram.tile([128, acols], F32, name=f"aslot{i}", tag=f"aslot{i}")
            for i in range(NSLOT)
        ]
        for sl in a_slots:
            nc.sync.dma_start(out=sl[:, :], in_=zero_sb[:])
        u_slots = [
            dram.tile([128, nb, k_scales], F32, name=f"uslot{i}", tag=f"uslot{i}")
            for i in range(NSLOT)
        ]

        # ---- group loop ----
        for g in range(n_groups):
            i0 = g * nb
            gnb = min(nb, n_tiles - i0)        # tiles in this group
            gt0 = i0 * TP
            has_partial = (gt0 + (gnb - 1) * TP + 128) > t_total or gnb < nb

            # -- merged x load: x_big[p, b, d] = x[gt0 + 125b + p, d] --
            x_big = xpool.tile([128, nb, d_total], F32)
            if has_partial:
                nc.gpsimd.memset(x_big[:], 0.0)
                for b in range(gnb):
                    t0 = gt0 + b * TP
                    rows = min(128, t_total - t0)
                    nc.sync.dma_start(
                        out=x_big[0:rows, b, :], in_=x_in.ap()[t0 : t0 + rows, :]
                    )
            else:
                x_src = bass.AP(
                    tensor=x_in.ap().tensor,
                    offset=gt0 * d_total,
                    ap=[[d_total, 128], [TP * d_total, gnb], [1, d_total]],
                )
                nc.sync.dma_start(out=x_big[:, 0:gnb, :], in_=x_src)
            if USE_F32R_BLEND:
                # round x to f32r in place (idle GpSimd) so the f32r blend
                # matmul sees a rounded producer; scores use the same values
                nc.gpsimd.tensor_copy(
                    x_big[:, :, :].bitcast(F32R), x_big[:, :, :]
                )

            u_big = upool.tile([128, nb, k_scales], F32)
            for b in range(gnb):
                # transposes: xT[d, t] per 128-chunk
                xt_psum = ppool_t.tile([128, d_total], F32)
                for c in range(n_chunks):
                    t_in = x_big[:, b, c * 128 : (c + 1) * 128]
                    t_id = identity[:, :]
                    t_out = xt_psum[:, c * 128 : (c + 1) * 128]
                    if USE_F32R_TRANSPOSE:
                        t_in = t_in.bitcast(F32R)
                        t_id = t_id.bitcast(F32R)
                        t_out = t_out.bitcast(F32R)
                    nc.tensor.transpose(t_out, t_in, t_id)
                xt_sb = xtpool.tile([128, d_total], F32)
                nc.scalar.copy(out=xt_sb[:], in_=xt_psum[:])

                # scores: u[t, k] = sum_d x[t, d] W[k, d]
                u_psum = ppool_u.tile([128, k_scales], F32)
                for c in range(n_chunks):
                    nc.tensor.matmul(
                        u_psum[:, :],
                        xt_sb[:, c * 128 : (c + 1) * 128],
                        w_sb[:, c, :],
                        start=(c == 0),
                        stop=(c == n_chunks - 1),
                    )
                nc.vector.tensor_copy(u_big[:, b, :], u_psum[:])

            # -- u roundtrip: 1 write + 3 shifted reads (partition shift) --
            uslot = u_slots[g % NSLOT]
            nc.sync.dma_start(out=uslot[:, 0:gnb, :], in_=u_big[:, 0:gnb, :])
            usl_ap = uslot[:, :, :]
            us_j = []
            for j in range(1, k_scales):
                usj = smalls.tile(
                    [128, nb, k_scales], F32, name=f"us{j}", tag=f"us{j}"
                )
                src = bass.AP(
                    tensor=usl_ap.tensor,
                    offset=usl_ap.offset + j * nb * k_scales,
                    ap=[
                        [nb * k_scales, TP],
                        [k_scales, gnb],
                        [1, k_scales],
                    ],
                )
                nc.sync.dma_start(out=usj[0:TP, 0:gnb, :], in_=src)
                us_j.append(usj)

            # -- per-tile smalls -> blend coefficients C --
            c_big = cpool.tile([128, k_scales, nb], F32)
            for b in range(gnb):
                i = i0 + b
                t0 = gt0 + b * TP
                cols = min(TP, t_total - t0)
                last = i == n_tiles - 1

                y = smalls.tile([128, k_scales], F32)
                nc.gpsimd.tensor_copy(y[0:TP, :], u_big[0:TP, b, :])
                for j in range(1, k_scales):
                    nc.gpsimd.tensor_add(
                        y[0:TP, j:k_scales],
                        y[0:TP, j:k_scales],
                        us_j[j - 1][0:TP, b, j:k_scales],
                    )
                if last:
                    # zero scores where the pooling window passes T
                    nc.gpsimd.affine_select(
                        out=y[0:TP, :],
                        in_=y[0:TP, :],
                        compare_op=mybir.AluOpType.is_ge,
                        fill=0.0,
                        base=cols - 1,
                        pattern=[[-1, k_scales]],
                        channel_multiplier=-1,
                    )

                e = smalls.tile([128, k_scales], F32)
                nc.scalar.activation(
                    e[0:TP, :], y[0:TP, :], mybir.ActivationFunctionType.Exp
                )
                z = smalls.tile([128, 1], F32)
                nc.vector.tensor_reduce(
                    z[0:TP, :], e[0:TP, :], axis=mybir.AxisListType.X,
                    op=mybir.AluOpType.add,
                )
                r = smalls.tile([128, 1], F32)
                nc.vector.reciprocal(r[0:TP, :], z[0:TP, :])

                gg = smalls.tile([128, k_scales], F32, name="gg", tag="gg")
                nc.vector.tensor_mul(gg[0:TP, :], e[0:TP, :], invk[0:TP, :])
                if last:
                    nc.gpsimd.affine_select(
                        out=gg[0:TP, :],
                        in_=gg[0:TP, :],
                        compare_op=mybir.AluOpType.is_ge,
                        fill=0.0,
                        base=cols - 1,
                        pattern=[[-1, k_scales]],
                        channel_multiplier=-1,
                    )
                for j in range(k_scales - 2, -1, -1):
                    nc.vector.tensor_add(
                        gg[0:TP, j : j + 1],
                        gg[0:TP, j : j + 1],
                        gg[0:TP, j + 1 : j + 2],
                    )
                nc.vector.tensor_scalar_mul(
                    c_big[0:TP, :, b], gg[0:TP, :], r[0:TP, :]
                )

            # -- one staircase write + one readback per group --
            # interleaved A layout: flat cell (t, t'*nb + b) so the b-dim is
            # contiguous; cell (t'+j, t', b) <- C[t', j, b]
            slot = a_slots[g % NSLOT]
            slot_ap = slot[:, :]
            for j in range(k_scales):
                stair = bass.AP(
                    tensor=slot_ap.tensor,
                    offset=slot_ap.offset + j * acols,
                    ap=[[acols + nb, TP], [1, gnb]],
                )
                nc.sync.dma_start(out=stair, in_=c_big[0:TP, j, 0:gnb])

            a_big = apool.tile([128, acols], F32)
            nc.sync.dma_start(out=a_big[:, :], in_=slot[:, :])
            if USE_F32R_BLEND:
                nc.gpsimd.tensor_copy(a_big[:, :].bitcast(F32R), a_big[:, :])

            # -- blend matmuls + PSUM->SBUF copies --
            o_big = opool.tile([128, nb, d_total], F32)
            for b in range(gnb):
                t0 = gt0 + b * TP
                cols = min(TP, t_total - t0)
                rows = min(128, t_total - t0)
                o_psum = ppool_o.tile([128, d_total], F32)
                a_r = a_big[:, :].rearrange("p (t x) -> p t x", x=nb)
                bl_a = a_r[0:rows, 0:cols, b]
                bl_x = x_big[0:rows, b, :]
                if USE_F32R_BLEND:
                    bl_a = bl_a.bitcast(F32R)
                    bl_x = bl_x.bitcast(F32R)
                nc.tensor.matmul(
                    o_psum[0:cols, :], bl_a, bl_x, start=True, stop=True
                )
                nc.scalar.copy(out=o_big[0:cols, b, 0:half], in_=o_psum[0:cols, 0:half])
                nc.vector.tensor_copy(
                    o_big[0:cols, b, half:], o_psum[0:cols, half:]
                )

            # -- merged out store --
            if has_partial:
                for b in range(gnb):
                    t0 = gt0 + b * TP
                    cols = min(TP, t_total - t0)
                    nc.scalar.dma_start(
                        out=out_dram.ap()[t0 : t0 + cols, :],
                        in_=o_big[0:cols, b, :],
                    )
            else:
                o_dst = bass.AP(
                    tensor=out_dram.ap().tensor,
                    offset=gt0 * d_total,
                    ap=[[d_total, TP], [TP * d_total, gnb], [1, d_total]],
                )
                nc.scalar.dma_start(out=o_dst, in_=o_big[0:TP, 0:gnb, :])

    nc.finalize()
    return nc


_NC_CACHE = {}


def _get_nc(t_total=T):
    if t_total not in _NC_CACHE:
        _NC_CACHE[t_total] = build_nc(t_total=t_total)
    return _NC_CACHE[t_total]


def run_spmd(x, W, trace=False, **spmd_kwargs):
    """x [B, T, D], W [K, D] -> (out [B, T, D], BassKernelResults)."""
    from concourse.bass_utils import run_bass_kernel_spmd

    x = np.ascontiguousarray(np.asarray(x, dtype=np.float32))
    W = np.ascontiguousarray(np.asarray(W, dtype=np.float32))
    assert x.shape == (B, T, D) and W.shape == (K, D), (x.shape, W.shape)
    nc = _get_nc()
    in_maps = [{"x": x[b], "W": W} for b in range(B)]
    res = run_bass_kernel_spmd(
        nc, in_maps, core_ids=list(range(N_CORES)), trace=trace, **spmd_kwargs
    )
    out = np.stack([r["out"] for r in res.results], axis=0)
    return out, res


def kernel(x, W, max_k=None, **_):
    out, _res = run_spmd(x, W)
    return out

